# revision 48
# baseline (speedup 1.0000x reference)
"""Trainium2 Bass kernel for nn_DTFOS: fractional differencing residual.

Per batch b (one per NeuronCore, 8 cores):
    Y = fracdiff(X, relu(alpha))      # causal conv with (1-L)^alpha weights
    E = Y[1:, :] - X[:-1, :] @ A.T

Algorithm: fracdiff weights decay as k^(-1-alpha); the kernel is truncated
to K=64 taps (validated rel err ~3e-3 vs the 2e-2 gate). The conv becomes
overlap-save with 128-sample windows, hop 64, in the ODD-FREQUENCY
(negacyclic) DFT basis: bins (f+1/2)*2pi/128, f=0..63. Real signals need
exactly 64 complex bins, and discarded wrap rows are exact linear conv.

Per window (one matmul each, stationary reused):
  [Zr;Zi] = CF^T @ xwin                 (PE: 128-contraction, stacked r/i)
  m_a = [Zr;Zi]*[Wr;Wi], m_b = [Zr;Zi]*[Wi;Wr]   (DVE, bf16 2x)
  E    = SA^T @ m_a + SB^T @ m_b + XTslice^T @ (-A^T)   (PE, PSUM accum)
Even/odd windows live in PSUM rows 0..63 / 64..127 (tile_position=(0,64));
the Yhat term and the +1 output shift enter via an X^T stationary offset
by one column. w taps are built on device from alpha (log-cumsum recurrence
via iota/scan/exp); the W spectra are replicated for DVE 2x-mode products.

Inputs per core: XR = X relayout [s,128-block,c] bf16, XT = X^T bf16 (both
pure host relayouts of X), alpha, A f32, one merged DFT-constant tensor.
Output E fp32. No DRAM scratch; ~6.3 MiB HBM in / 4 MiB out per core.

kernel(**inputs) takes FULL inputs (8, 8192, 128)/(8, 128)/(8, 128, 128),
shards batch over 8 cores, returns FULL output (8, 8191, 128) fp32.
"""
import sys
import numpy as np

sys.path.insert(0, "/opt/trn_rl_repo")

import ml_dtypes  # noqa: E402
from contextlib import ExitStack  # noqa: E402

import concourse.bass as bass  # noqa: E402
import concourse.mybir as mybir  # noqa: E402
import concourse.tile as tile  # noqa: E402
from concourse.masks import make_identity  # noqa: E402

F32 = mybir.dt.float32
BF16 = mybir.dt.bfloat16
AF = mybir.ActivationFunctionType
OP = mybir.AluOpType

T = 8192          # time steps
NCH = 128         # channels per core
NB = 64           # overlap-save windows (hop 128)
KTAP = 64         # truncated fracdiff taps
NQ = 4            # pipeline quarters
UQ = 16           # u-blocks (128-sample spans) per quarter
SG = 4            # u-blocks per matmul subgroup (free dim 512)


def _host_consts():
    bf = ml_dtypes.bfloat16
    L = 128.0
    s = np.arange(128, dtype=np.float64)[:, None]
    fh = np.arange(64, dtype=np.float64)[None, :] + 0.5
    th = 2.0 * np.pi * fh * s / L                          # [s, f]
    CF = np.concatenate([np.cos(th), -np.sin(th)], axis=1)  # [s, 128]
    CW = np.concatenate([-np.sin(th[:64]), np.cos(th[:64])], axis=1)  # [k, 128]
    rt = np.arange(64, dtype=np.float64)[None, :] + 64.0
    thI = 2.0 * np.pi * fh.T * rt / L                      # [f, rt]
    IRc = (2.0 / L) * np.cos(thI)
    IIc = -(2.0 / L) * np.sin(thI)
    SA = np.concatenate([IRc, -IRc], axis=0)               # [128, 64]
    SB = np.concatenate([IIc, IIc], axis=0)                # [128, 64]
    CWp = np.zeros((128, 128))
    CWp[:64] = CW
    consts = {"CC": np.concatenate(
        [CF, np.concatenate([SA, SB], axis=1), CWp], axis=1).astype(bf)}
    return consts


_CONSTS = _host_consts()


def build_program(split_waits=True):
    nc = bass.Bass()
    xr_h = nc.declare_dram_parameter("XR", [128, NB, NCH], BF16, isOutput=False)
    xt_h = nc.declare_dram_parameter("XT", [NCH, T], BF16, isOutput=False)
    al_h = nc.declare_dram_parameter("alpha", [NCH, 1], F32, isOutput=False)
    a_h = nc.declare_dram_parameter("A", [NCH, NCH], F32, isOutput=False)
    ch_: dict[str, bass.AP] = {}
    for name, arr in _CONSTS.items():
        dt = F32 if arr.dtype == np.float32 else BF16
        ch_[name] = nc.declare_dram_parameter(name, list(arr.shape), dt, isOutput=False)
    e_h = nc.declare_dram_parameter("E", [T - 1, NCH], F32, isOutput=True)

    hw = nc.hwdge_engines
    dmae = [getattr(nc, e.name.lower(), None) for e in hw] if hw else [nc.sync]
    dmae = [e for e in dmae if e is not None] or [nc.sync]

    def dma(i, out, in_):
        eng = dmae[i % len(dmae)]
        with nc.allow_non_contiguous_dma(reason="layout"):
            eng.dma_start(out=out, in_=in_)

    with tile.TileContext(nc) as tc, ExitStack() as ctx:
        consts = ctx.enter_context(tc.tile_pool(name="consts", bufs=1))
        wp = ctx.enter_context(tc.tile_pool(name="wp", bufs=1))
        psA = ctx.enter_context(tc.tile_pool(name="psA", bufs=3, space="PSUM"))
        psE = ctx.enter_context(tc.tile_pool(name="psE", bufs=2, space="PSUM"))
        mtmp = ctx.enter_context(tc.tile_pool(name="mtmp", bufs=2))
        eep = ctx.enter_context(tc.tile_pool(name="eep", bufs=3))

        # ---- persistent SBUF data ----
        data = ctx.enter_context(tc.tile_pool(name="data", bufs=1))
        xa = data.tile([128, 64, NCH], BF16, tag="xa")   # [s, u, c]: X[128u+s]
        xe = data.tile([128, 64, NCH], BF16, tag="xe")   # X[128u-64+s]
        xt = data.tile([128, 16 + T], BF16, tag="xt")    # [c, t+16]
        zf = data.tile([128, 2, 64, NCH], BF16, tag="zf")  # [fstack, par, u, c]
        wa = data.tile([128, NCH], BF16, tag="wa")       # [Wr;Wi] stacked
        wb = data.tile([128, NCH], BF16, tag="wb")       # [Wi;Wr] stacked
        waR = data.tile([128, UQ, NCH], BF16, tag="waR")
        wbR = data.tile([128, UQ, NCH], BF16, tag="wbR")
        nat = data.tile([128, NCH], BF16, tag="nat")     # [c, c'] = -A^T

        # ---- bulk loads: few big DMAs. sync queue: X views + XT.
        # scalar queue: merged consts + smalls + dummy Ln (ACT table). ----
        nc.vector.memset(xe[0:64, 0, :], 0.0)
        nc.vector.memset(xt[:, 0:16], 0.0)
        xv = xr_h[:]                                     # [s, m, c]
        # HAM warmup: dummy matmuls keep PE busy during the DMA window
        wrm = consts.tile([128, 512], BF16, tag="wrm")
        nc.vector.memset(wrm[:], 0.0)
        pwrm = psE.tile([128, SG * NCH], F32, tag="pse", name="pwrm")
        for _ in range(16):
            nc.tensor.matmul(pwrm[:], wrm[:, 0:128], wrm[:], start=True,
                             stop=True)
        ccs = consts.tile([128, 384], BF16, tag="ccs")
        nc.scalar.dma_start(out=ccs, in_=ch_["CC"][:])
        cF = ccs[:, 0:128]
        cSA = ccs[:, 128:192]
        cSB = ccs[:, 192:256]
        cCW = ccs[0:64, 256:384]
        for qq in range(2):
            u0 = qq * 32
            nc.sync.dma_start(out=xa[:, u0:u0 + 32, :], in_=xv[:, u0:u0 + 32, :])
            nc.sync.dma_start(out=xe[64:128, u0:u0 + 32, :],
                              in_=xv[0:64, u0:u0 + 32, :])
            lo = max(1, u0)
            nc.sync.dma_start(out=xe[0:64, lo:u0 + 32, :],
                              in_=xv[64:128, lo - 1:u0 + 32 - 1, :])
        nc.sync.dma_start(out=xt[:, 16:16 + T], in_=xt_h[:])
        ident = consts.tile([128, 128], F32, tag="ident")
        make_identity(nc, ident[:])

        # small input DMAs (scalar queue) + early ACT table load
        alr = wp.tile([NCH, 1], F32, tag="alr")
        nc.scalar.dma_start(out=alr, in_=al_h[:])
        an = wp.tile([NCH, NCH], F32, tag="an")
        nc.scalar.dma_start(out=an, in_=a_h[:])
        dum = wp.tile([1, 1], F32, tag="dum")
        nc.vector.memset(dum[:], 1.0)
        nc.scalar.activation(dum[:], dum[:], AF.Ln)
        # k tables on device (no DMA dependency): iota -> f32
        ki32 = wp.tile([NCH, KTAP], mybir.dt.int32, tag="ki32")
        nc.gpsimd.iota(ki32[:], [[1, KTAP]], channel_multiplier=0)
        kf = wp.tile([NCH, KTAP], F32, tag="kf")
        nc.vector.tensor_copy(kf[:], ki32[:])

        def build_w():
            # ---- w taps (64) -> stacked spectra [Wr;Wi], [Wi;Wr] ----
            nc.vector.tensor_scalar_max(alr[:], alr[:], 0.0)
            lga = wp.tile([NCH, 1], F32, tag="lga")
            nc.scalar.activation(lga[:], alr[:], AF.Ln)
            alr1 = wp.tile([NCH, 1], F32, tag="alr1")
            nc.vector.tensor_scalar_add(alr1[:], alr[:], 1.0)
            t1 = wp.tile([NCH, KTAP], F32, tag="t1")
            nc.vector.tensor_scalar(out=t1[:], in0=kf[:], scalar1=alr1[:],
                                    scalar2=None, op0=OP.subtract)
            nc.vector.tensor_scalar_max(t1[:], t1[:], 1e-30)
            nc.scalar.activation(t1[:], t1[:], AF.Ln)    # ln(k-1-alpha)
            nc.vector.memset(t1[:, 0:2], 0.0)
            cum = wp.tile([NCH, KTAP], F32, tag="cum")
            nc.vector.tensor_tensor_scan(out=cum[:], data0=t1[:], data1=t1[:],
                                         initial=0.0, op0=OP.add, op1=OP.bypass)
            ctb = wp.tile([NCH, KTAP], F32, tag="ctb")
            nc.vector.tensor_scalar_max(ctb[:], kf[:], 1.0)
            nc.scalar.activation(ctb[:], ctb[:], AF.Ln)
            nc.vector.tensor_tensor_scan(out=ctb[:], data0=ctb[:], data1=ctb[:],
                                         initial=0.0, op0=OP.add, op1=OP.bypass)
            nc.vector.tensor_sub(cum[:], cum[:], ctb[:])
            nc.vector.tensor_scalar(out=cum[:], in0=cum[:], scalar1=lga[:],
                                    scalar2=None, op0=OP.add)
            wch = wp.tile([NCH, KTAP], F32, tag="wch")
            nc.scalar.activation(wch[:], cum[:], AF.Exp, scale=1.0)
            negone = wp.tile([NCH, 1], F32, tag="negone")
            nc.vector.memset(negone[:], -1.0)
            nc.vector.tensor_tensor(out=wch[:], in0=wch[:],
                                    in1=negone[:].to_broadcast([NCH, KTAP]),
                                    op=OP.mult)
            nc.vector.memset(wch[:, 0:1], 1.0)
            pw = psE.tile([128, 128], F32, tag="pse", name="pw")
            nc.tensor.transpose(pw[0:KTAP, :], wch[:], ident[:])
            wkc = wp.tile([KTAP, NCH], BF16, tag="wkc")
            nc.scalar.activation(wkc[:], pw[0:KTAP, :], AF.Copy)
            pwa = psE.tile([128, NCH], F32, tag="pse", name="pwa")
            nc.tensor.matmul(pwa[:], cF[0:KTAP, :], wkc[:], start=True, stop=True)
            nc.scalar.activation(wa[:], pwa[:], AF.Copy)
            pwb = psE.tile([128, NCH], F32, tag="pse", name="pwb")
            nc.tensor.matmul(pwb[:], cCW, wkc[:], start=True, stop=True)
            nc.scalar.activation(wb[:], pwb[:], AF.Copy)
            for wi_, (wsrc, wdst) in enumerate(((wa, waR), (wb, wbR))):
                srcb = wsrc[:].rearrange("f (u c) -> f u c", u=1).to_broadcast(
                    [128, UQ, NCH])
                if wi_ == 0:
                    nc.scalar.activation(wdst[:], srcb, AF.Copy)
                else:
                    nc.vector.tensor_copy(wdst[:], srcb)
            # ---- -A^T ----
            pa = psE.tile([128, 128], F32, tag="pse", name="pa")
            nc.tensor.transpose(pa[:], an[:], ident[:])
            nc.scalar.activation(nat[:], pa[:], AF.Copy, scale=-1.0)

        def phase_a(q):
            # forward DFT: one 128-contraction matmul per 4 windows, output
            # is the stacked [Zr;Zi] spectrum; same CF stationary throughout
            for sub in range(UQ // SG):
                u0 = q * UQ + sub * SG
                px = psA.tile([128, 2, SG * NCH], F32, tag="px", name="px")
                nc.tensor.matmul(px[:, 0, :], cF, xe[:, u0:u0 + SG, :],
                                 start=True, stop=True)
                nc.tensor.matmul(px[:, 1, :], cF, xa[:, u0:u0 + SG, :],
                                 start=True, stop=True)
                src_ap = px[:].rearrange("f p (u c) -> f p u c", c=NCH)
                nc.scalar.activation(zf[:, :, u0:u0 + SG, :], src_ap, AF.Copy)

        def phase_b(q):
            # stacked spectrum products: 4 big DVE multiplies per quarter
            sl = slice(q * UQ, (q + 1) * UQ)
            ms = [mtmp.tile([128, UQ, NCH], BF16, tag=f"m{i}", name=f"m{i}")
                  for i in range(4)]
            nc.vector.tensor_mul(ms[0][:], zf[:, 0, sl, :], waR[:])  # even a
            nc.vector.tensor_mul(ms[1][:], zf[:, 0, sl, :], wbR[:])  # even b
            nc.vector.tensor_mul(ms[2][:], zf[:, 1, sl, :], waR[:])  # odd a
            nc.vector.tensor_mul(ms[3][:], zf[:, 1, sl, :], wbR[:])  # odd b
            pk = psA.tile([128, 2, SG * NCH], F32, tag="px", name="pk")
            nc.tensor.matmul(pk[:, 0, :], cF, ms[0][:, 0:SG, :],
                             start=True, stop=True)
            return ms

        def phase_c(q, ms):
            # inverse DFT + Yhat accumulated in PSUM: even windows in rows
            # 0..63, odd windows in rows 64..127 (tile_position=(0,64))
            ee = eep.tile([128, UQ * NCH], F32, tag="ee")
            for sub in range(UQ // SG):
                u0 = q * UQ + sub * SG
                gl = slice(sub * SG, sub * SG + SG)
                pse = psE.tile([128, SG * NCH], F32, tag="pse", name="pse")
                nc.tensor.matmul(pse[0:64, :], cSA, ms[0][:, gl, :],
                                 start=True, stop=False)
                nc.tensor.matmul(pse[0:64, :], cSB, ms[1][:, gl, :],
                                 start=False, stop=False)
                for w2 in range(SG):
                    u = u0 + w2
                    nc.tensor.matmul(
                        pse[0:64, w2 * NCH:(w2 + 1) * NCH],
                        xt[:, 15 + 128 * u: 15 + 128 * u + 64],
                        nat[:], start=False, stop=(w2 == SG - 1))
                nc.tensor.matmul(pse[64:128, :], cSA, ms[2][:, gl, :],
                                 start=True, stop=False, tile_position=(0, 64))
                nc.tensor.matmul(pse[64:128, :], cSB, ms[3][:, gl, :],
                                 start=False, stop=False, tile_position=(0, 64))
                for w2 in range(SG):
                    u = u0 + w2
                    nc.tensor.matmul(
                        pse[64:128, w2 * NCH:(w2 + 1) * NCH],
                        xt[:, 79 + 128 * u: 79 + 128 * u + 64],
                        nat[:], start=False, stop=(w2 == SG - 1),
                        tile_position=(0, 64))
                esl = ee[:, sub * SG * NCH:(sub + 1) * SG * NCH]
                if sub % 2 == 0:
                    nc.scalar.activation(esl, pse[:], AF.Copy)
                else:
                    nc.vector.tensor_copy(esl, pse[:])
            u0q = q * UQ
            eev = ee[:].rearrange("r (u c) -> r u c", c=NCH)
            if q == 0:
                dma(0, e_h[0:63, :], ee[1:64, 0:NCH])
                dma(1, e_h[63:127, :], ee[64:128, 0:NCH])
                ov = e_h[127:127 + (UQ - 1) * 128, :].rearrange(
                    "(u p r) c -> (p r) u c", p=2, r=64)
                dma(2, ov, eev[:, 1:UQ, :])
            elif q == NQ - 1:
                for s2 in range(0, UQ, SG):
                    us = u0q + s2
                    if s2 < UQ - SG:
                        ov = e_h[us * 128 - 1: us * 128 - 1 + SG * 128,
                                 :].rearrange("(u p r) c -> (p r) u c",
                                              p=2, r=64)
                        dma(s2, ov, eev[:, s2:s2 + SG, :])
                    else:
                        # final write split per u-block: the E row pattern is
                        # 512B-scattered (descriptor-bound), so keep the very
                        # last DMA's descriptor count small to shrink the
                        # exit-drain straggler
                        for u4 in range(SG):
                            uu = us + u4
                            ov = e_h[uu * 128 - 1: uu * 128 + 127,
                                     :].rearrange("(u p r) c -> (p r) u c",
                                                  p=2, r=64)
                            dma(u4, ov, eev[:, s2 + u4:s2 + u4 + 1, :])
            else:
                ov = e_h[u0q * 128 - 1: u0q * 128 - 1 + UQ * 128,
                         :].rearrange("(u p r) c -> (p r) u c", p=2, r=64)
                dma(q, ov, eev)

        # software-pipelined emission: PE starts on the forward DFT right
        # after the first X chunk lands; the W-spectrum build overlaps it;
        # PE always has quarter q+2's forward DFT during quarter q's products
        build_w()
        phase_a(0)
        phase_a(1)
        prev = None
        for q in range(NQ):
            ms = phase_b(q)
            if prev is not None:
                phase_c(*prev)
            if q + 2 < NQ:
                phase_a(q + 2)
            prev = (q, ms)
        phase_c(*prev)

    if split_waits:
        _split_waits(nc)
    return nc


def _split_waits(nc):
    """Walrus allows 1 inline sem-wait per compute instruction (2 per DMA).
    Hoist excess waits into standalone EventSemaphore instructions on the
    same engine right before the instruction (semantically identical)."""
    caps = {}
    n_split = 0
    for fn in nc.m.functions:
        for blk in fn.blocks:
            out = []
            for ins in blk.instructions:
                si = getattr(ins, "sync_info", None)
                waits = list(si.on_wait) if si is not None and si.on_wait else []
                cap = caps.get(str(ins.opcode), 1)
                if len(waits) > cap:
                    for k, w in enumerate(waits[:-cap]):
                        es = mybir.InstEventSemaphore(
                            name=f"wsp_{ins.name}_{k}")
                        es.engine = ins.engine
                        es.sync_info = mybir.SyncInfo(on_wait=[w], on_update=[])
                        out.append(es)
                        n_split += 1
                    si.on_wait = waits[-cap:]
                out.append(ins)
            blk.instructions = out
    return n_split


_NC = None


def _get_nc(split_waits=True):
    global _NC
    if _NC is None:
        _NC = build_program(split_waits=split_waits)
    return _NC


def kernel(X, alpha, A):
    from concourse.bass_utils import run_bass_kernel_spmd
    nc = _get_nc()
    B = X.shape[0]
    core_ids = list(range(B))
    in_maps = []
    for b in range(B):
        m = {"X": np.ascontiguousarray(X[b], dtype=np.float32),
             "XT": np.ascontiguousarray(X[b].T, dtype=np.float32),
             "alpha": np.ascontiguousarray(alpha[b].reshape(NCH, 1), dtype=np.float32),
             "A": np.ascontiguousarray(A[b], dtype=np.float32)}
        for name, arr in _CONSTS.items():
            m[name] = arr
        in_maps.append(m)
    res = run_bass_kernel_spmd(nc, in_maps, core_ids)
    out = np.stack([res.results[b]["E"] for b in range(B)], axis=0)
    return out.astype(np.float32)
